# revision 1
# baseline (speedup 1.0000x reference)
"""ContrastiveLoss (margin=1) on 8 trn2 NeuronCores via Bass/Tile.

Math: with d = cdist(output1, output2) [N, M], pos_r = rowmin(d),
pos_c = colmin(d), every hinge term  margin - pos + d >= margin > 0,
and the excluded (argmin) entry equals exactly margin.  Hence

  image_losses.mean() = 1 - mean(pos_r) - 1/M + sum(d)/(N*M)
  text_losses.mean()  = 1 - mean(pos_c) - 1/N + sum(d)/(N*M)
  loss = (1 - 1/N) + sum(d)/(N*M) - (mean(pos_r) + mean(pos_c))/2      (N == M)

So the kernel only needs sum(d), rowmin(d), colmin(d): one pass over the
distance matrix.  Sharding: core c owns rows [c*1024, (c+1)*1024) of
output1 and all of output2; colmin partials are combined with an
all-reduce(min), the scalar partials with an all-reduce(add).
"""

import numpy as np
from contextlib import ExitStack

N = 8192          # rows of output1 == rows of output2
D = 128           # feature dim (== max matmul contraction)
NCORES = 8
R = N // NCORES   # 1024 rows per core
JT = 512          # free-dim tile (one PSUM bank of fp32)
NJT = N // JT     # 16 j-tiles
NIB = R // 128    # 8 row blocks per core

MARGIN = 1.0
C0 = 1.0 / (float(N) * float(N))      # scale for sum(d)
C1 = -1.0 / (2.0 * float(N))          # scale for sum(pos_r)
C2 = -1.0 / (2.0 * float(N))          # scale for sum(pos_c)
CONST = MARGIN - MARGIN / float(N)    # 1 - 1/8192

_CACHE = {}


def _build():
    import concourse.bass as bass
    import concourse.bacc as bacc
    import concourse.tile as tile
    from concourse import mybir
    from concourse import bass_isa

    f32 = mybir.dt.float32
    f32r = mybir.dt.float32r
    bf16 = mybir.dt.bfloat16
    X = mybir.AxisListType.X
    MIN = mybir.AluOpType.min
    ADD = mybir.AluOpType.add
    MULT = mybir.AluOpType.mult
    Sqrt = mybir.ActivationFunctionType.Sqrt

    # Bacc (not raw Bass): its compile() runs move_matmul_waits_to_ldweights
    # + generate_event_semaphores, which legalize multi-semaphore waits down
    # to the 1-wait-per-instruction TRN2 ISA budget.
    nc = bacc.Bacc(
        trn_type="TRN2",
        target_bir_lowering=False,
        debug=False,
        num_devices=NCORES,
    )

    # Flipped orientation: each core owns a 1024-row strip of output2 (b)
    # and sees all of output1 (a).  It computes e = dist(b_strip, a_full)
    # [1024, 8192]: e^2 = r2[j] + r1[i] - 2 b a^T, with j on partitions.
    # r2[j] is a per-partition ACT-bias; r1[i] is a K=1 rank-1 matmul whose
    # operands are partition-0 rows (PE LDWEIGHTS carries at most ONE
    # semaphore wait, so every PE operand is produced by a single engine:
    # ACT for matmul operands, DVE for transpose inputs).
    a_ext = nc.dram_tensor("a", [N, D], f32, kind="ExternalInput")
    b_ext = nc.dram_tensor("b", [R, D], f32, kind="ExternalInput")
    # per-core one-hot mask row: 0.0 at this core's slot, 1e30 elsewhere --
    # lets the min-all-reduce double as an all-gather of per-core scalars.
    cmask_ext = nc.dram_tensor("cmask", [1, NCORES], f32, kind="ExternalInput")
    out_ext = nc.dram_tensor("out", [1, 1], f32, kind="ExternalOutput")

    groups = [list(range(NCORES))]

    with tile.TileContext(nc) as tc, ExitStack() as ctx:
        const = ctx.enter_context(tc.tile_pool(name="const", bufs=1))
        big = ctx.enter_context(tc.tile_pool(name="big", bufs=1))
        stage = ctx.enter_context(tc.tile_pool(name="stage", bufs=3))
        dpool = ctx.enter_context(tc.tile_pool(name="dpool", bufs=6))
        tpsum = ctx.enter_context(tc.tile_pool(name="tpsum", bufs=2, space="PSUM"))
        rpsum = ctx.enter_context(tc.tile_pool(name="rpsum", bufs=2, space="PSUM"))
        mpsum = ctx.enter_context(tc.tile_pool(name="mpsum", bufs=4, space="PSUM"))
        dram = ctx.enter_context(tc.tile_pool(name="dram", bufs=1, space="DRAM"))

        id_dram = nc.inline_tensor(np.eye(128, dtype=np.float32), name="id128")
        identityd = const.tile([128, 128], f32)
        nc.sync.dma_start(out=identityd, in_=id_dram[:, :])
        identity = const.tile([128, 128], f32)
        nc.vector.tensor_copy(out=identity, in_=identityd)
        identity_bf = const.tile([128, 128], bf16)
        nc.vector.tensor_copy(out=identity_bf, in_=identityd)

        # f32r constants, produced by engine rounding (not raw memset bits)
        ones128f = const.tile([128, 1], f32)
        nc.vector.memset(ones128f, 1.0)
        ones128 = const.tile([128, 1], f32r)
        nc.scalar.copy(out=ones128, in_=ones128f)
        onesrf = const.tile([1, 128], f32)
        nc.vector.memset(onesrf, 1.0)
        ones_row = const.tile([1, 128], f32r)
        nc.scalar.copy(out=ones_row, in_=onesrf)

        # single big DMAs (DMA trigger slots also carry only one wait, so
        # avoid per-tile DMA slot reuse entirely)
        # b first (the b-loop runs first and is small), then a in two
        # halves so early a-tiles are available at ~half the DMA latency.
        b_nat = big.tile([128, NIB, D], f32)
        nc.sync.dma_start(
            out=b_nat, in_=b_ext[:, :].rearrange("(q p) d -> p q d", p=128))
        a_nat = big.tile([128, N // 128, D], f32)
        H = N // 256
        nc.sync.dma_start(
            out=a_nat[:, :H, :],
            in_=a_ext[:N // 2, :].rearrange("(q p) d -> p q d", p=128))
        nc.sync.dma_start(
            out=a_nat[:, H:, :],
            in_=a_ext[N // 2:, :].rearrange("(q p) d -> p q d", p=128))

        # ---- b strip: m2bT = -2 * b^T (f32r); r2_vec [128, NIB] via DVE ----
        m2bT = big.tile([128, R], f32r)
        r2_vec = const.tile([128, NIB], f32)
        for q in range(NIB):
            bnat2 = stage.tile([128, D], f32, tag="stage_nat")
            nc.vector.tensor_copy(out=bnat2, in_=b_nat[:, q, :])
            pst = tpsum.tile([128, 128], f32, tag="tps")
            nc.tensor.transpose(pst, bnat2, identity)
            nc.vector.tensor_scalar_mul(m2bT[:, q * 128:(q + 1) * 128], pst, -2.0)
            scr = stage.tile([128, D], f32, tag="stage_scr")
            nc.vector.scalar_tensor_tensor(
                out=scr, in0=bnat2, scalar=1.0, in1=bnat2,
                op0=mybir.AluOpType.mult, op1=MULT,
                accum_out=r2_vec[:, q:q + 1])

        # ---- a full: aT = a^T (f32r); r1_row [1, N] via ones-matmul ----
        aT = big.tile([128, N], f32r)
        r1_row = big.tile([1, N], f32r)
        for q in range(N // 128):
            anat2 = stage.tile([128, D], f32, tag="stage_nat")
            nc.vector.tensor_copy(out=anat2, in_=a_nat[:, q, :])
            pst = tpsum.tile([128, 128], f32, tag="tps")
            nc.tensor.transpose(pst, anat2, identity)
            nc.vector.tensor_copy(out=aT[:, q * 128:(q + 1) * 128], in_=pst)
            sq = stage.tile([128, 128], f32r, tag="stage_sq")
            # pst * aT == pst^2 (aT holds the same values, DVE-written just
            # above) -- keeps the square off ACT with only one PSUM operand.
            nc.vector.scalar_tensor_tensor(
                out=sq, in0=pst, scalar=1.0,
                in1=aT[:, q * 128:(q + 1) * 128],
                op0=MULT, op1=MULT)
            prr = rpsum.tile([1, 128], f32, tag="rps")
            nc.tensor.matmul(prr, lhsT=ones128, rhs=sq, start=True, stop=True)
            nc.scalar.copy(out=r1_row[0:1, q * 128:(q + 1) * 128], in_=prr)

        # ---- main pass over e^2 tiles [128, 512] ----
        dsum_all = big.tile([128, NIB * NJT], f32)     # per-tile sum of e
        colminacc = big.tile([128, N], bf16)            # e-space col-min partials
        rowmin8 = const.tile([128, NIB], bf16)          # per-block row mins
        colmin_t = const.tile([128, N // 128], f32)

        rmpool = ctx.enter_context(tc.tile_pool(name="rmpool", bufs=2))
        for jb in range(NIB):
            wA = m2bT[:, jb * 128:(jb + 1) * 128]
            bias = r2_vec[:, jb:jb + 1]
            rowminacc = rmpool.tile([128, JT], bf16, tag="rma")
            for it in range(NJT):
                sl = slice(it * JT, (it + 1) * JT)
                ps = mpsum.tile([128, JT], f32, tag="mps")
                nc.tensor.matmul(ps, lhsT=wA, rhs=aT[:, sl],
                                 start=True, stop=False)
                nc.tensor.matmul(ps, lhsT=ones_row, rhs=r1_row[0:1, sl],
                                 start=False, stop=True)
                s = jb * NJT + it
                dsc = dpool.tile([128, JT], bf16, tag="dsc")
                nc.scalar.activation(
                    out=dsc, in_=ps, func=Sqrt, bias=bias, scale=1.0,
                    accum_out=dsum_all[:, s:s + 1])
                if it == 0:
                    nc.vector.tensor_copy(out=rowminacc, in_=dsc)
                else:
                    nc.vector.tensor_tensor(
                        out=rowminacc, in0=dsc, in1=rowminacc, op=MIN)
                if jb == 0:
                    nc.vector.tensor_copy(out=colminacc[:, sl], in_=dsc)
                else:
                    nc.vector.tensor_tensor(
                        out=colminacc[:, sl], in0=dsc, in1=colminacc[:, sl],
                        op=MIN)
                if jb == NIB - 1:
                    # col-min over this i-range is final: partition-reduce now
                    # so the min-all-reduce can start as the loop drains.
                    for q in range(it * (JT // 128), (it + 1) * (JT // 128)):
                        pst = tpsum.tile([128, 128], bf16, tag="tps")
                        nc.tensor.transpose(
                            pst, colminacc[:, q * 128:(q + 1) * 128],
                            identity_bf)
                        nc.vector.tensor_reduce(
                            out=colmin_t[:, q:q + 1], in_=pst, axis=X, op=MIN)
            nc.vector.tensor_reduce(
                out=rowmin8[:, jb:jb + 1], in_=rowminacc, axis=X, op=MIN)

        # ---- local scalar stats + all-reduce(add) ----
        # rowmin_all rows are pos_c for this core's own j-strip (complete).
        dsum_vec = const.tile([128, 1], f32)
        nc.vector.tensor_reduce(out=dsum_vec, in_=dsum_all, axis=X, op=ADD)
        posc_vec = const.tile([128, 1], f32)
        nc.vector.tensor_reduce(out=posc_vec, in_=rowmin8, axis=X, op=ADD)
        dsum_sc = const.tile([128, 1], f32)
        nc.vector.tensor_scalar_mul(dsum_sc, dsum_vec, C0)
        combo_l = const.tile([128, 1], f32)
        nc.vector.scalar_tensor_tensor(
            out=combo_l, in0=posc_vec, scalar=C2, in1=dsum_sc,
            op0=MULT, op1=ADD)

        # combo scalars ride along the min-all-reduce in NCORES extra
        # columns: this core's slot holds combo_l, other slots +1e30.
        cmaskd = const.tile([128, NCORES], f32)
        nc.sync.dma_start(
            out=cmaskd, in_=cmask_ext[0:1, :].to_broadcast((128, NCORES)))
        cmx = const.tile([128, NCORES], f32)
        nc.vector.tensor_scalar_add(cmx, cmaskd, combo_l)

        # ---- single all-reduce(min): [colmin_t | per-core combo slots] ----
        W = N // 128 + NCORES
        cm_in = dram.tile([128, W], f32)
        cm_out = dram.tile([128, W], f32)
        nc.sync.dma_start(out=cm_in[:, :N // 128], in_=colmin_t)
        nc.sync.dma_start(out=cm_in[:, N // 128:], in_=cmx)
        nc.gpsimd.collective_compute(
            "AllReduce", MIN, replica_groups=groups,
            ins=[cm_in.opt()], outs=[cm_out.opt()])
        colmin_g = const.tile([128, W], f32)
        nc.sync.dma_start(out=colmin_g, in_=cm_out)
        posr_vec = const.tile([128, 1], f32)
        nc.vector.tensor_reduce(
            out=posr_vec, in_=colmin_g[:, :N // 128], axis=X, op=ADD)
        combo_g = const.tile([128, 1], f32)
        nc.vector.tensor_reduce(
            out=combo_g, in_=colmin_g[:, N // 128:], axis=X, op=ADD)

        # ---- final combine ----
        total_vec = const.tile([128, 1], f32)
        nc.vector.scalar_tensor_tensor(
            out=total_vec, in0=posr_vec, scalar=C1, in1=combo_g,
            op0=MULT, op1=ADD)
        pr = const.tile([128, 1], f32)
        nc.gpsimd.partition_all_reduce(
            out_ap=pr, in_ap=total_vec, channels=128,
            reduce_op=bass_isa.ReduceOp.add)
        fin = const.tile([1, 1], f32)
        cbias = const.tile([1, 1], f32)
        nc.vector.memset(cbias, CONST)
        nc.scalar.activation(
            out=fin, in_=pr[0:1, :],
            func=mybir.ActivationFunctionType.Identity,
            bias=cbias, scale=1.0)
        nc.sync.dma_start(out=out_ext[:], in_=fin)

    if not nc.is_finalized():
        nc.finalize()
    return nc


def _get_nc():
    if "nc" not in _CACHE:
        _CACHE["nc"] = _build()
    return _CACHE["nc"]


def _in_maps(output1, output2):
    a = np.ascontiguousarray(np.asarray(output1, dtype=np.float32))
    b = np.ascontiguousarray(np.asarray(output2, dtype=np.float32))
    assert a.shape == (N, D) and b.shape == (N, D)
    masks = np.full((NCORES, 1, NCORES), 1e30, dtype=np.float32)
    for c in range(NCORES):
        masks[c, 0, c] = 0.0
    return [{"a": a, "b": b[c * R:(c + 1) * R], "cmask": masks[c]}
            for c in range(NCORES)]


def _run(output1, output2, trace=False):
    from concourse.bass_utils import run_bass_kernel_spmd

    res = run_bass_kernel_spmd(
        _get_nc(), _in_maps(output1, output2), list(range(NCORES)), trace=trace)
    out = np.asarray(res.results[0]["out"], dtype=np.float32).reshape(())
    return out, res


def kernel(output1, output2):
    out, _ = _run(output1, output2, trace=False)
    return out


# ---------------------------------------------------------------------------
# cached fast runner (mirrors bass2jax.run_bass_via_pjrt, but keeps the
# jitted sharded callable alive so repeated calls don't re-trace) — used by
# test.py for warm timing loops.
def _get_fast_runner():
    if "runner" in _CACHE:
        return _CACHE["runner"]

    import jax
    from jax.experimental.shard_map import shard_map
    from jax.sharding import Mesh, PartitionSpec
    from concourse import bass2jax, mybir

    nc = _get_nc()
    bass2jax.install_neuronx_cc_hook()

    partition_name = (
        nc.partition_id_tensor.name if nc.partition_id_tensor else None)
    in_names, out_names, out_avals = [], [], []
    for alloc in nc.m.functions[0].allocations:
        if not isinstance(alloc, mybir.MemoryLocationSet):
            continue
        name = alloc.memorylocations[0].name
        if alloc.kind == "ExternalInput":
            if name != partition_name:
                in_names.append(name)
        elif alloc.kind == "ExternalOutput":
            out_names.append(name)
            out_avals.append(jax.core.ShapedArray(
                tuple(alloc.tensor_shape), mybir.dt.np(alloc.dtype)))
    n_params = len(in_names)
    all_in_names = list(in_names) + list(out_names)
    if partition_name is not None:
        all_in_names.append(partition_name)

    def _body(*args):
        operands = list(args)
        if partition_name is not None:
            operands.append(bass2jax.partition_id_tensor())
        return tuple(bass2jax._bass_exec_p.bind(
            *operands,
            out_avals=tuple(out_avals),
            in_names=tuple(all_in_names),
            out_names=tuple(out_names),
            lowering_input_output_aliases=(),
            sim_require_finite=True,
            sim_require_nnan=True,
            nc=nc,
        ))

    devices = jax.devices()[:NCORES]
    mesh = Mesh(np.asarray(devices), ("core",))
    n_outs = len(out_names)
    sharded = jax.jit(
        shard_map(
            _body, mesh=mesh,
            in_specs=(PartitionSpec("core"),) * (n_params + n_outs),
            out_specs=(PartitionSpec("core"),) * n_outs,
            check_rep=False,
        ),
        keep_unused=True,
    )

    in_sharding = jax.sharding.NamedSharding(mesh, PartitionSpec("core"))

    def prep(in_maps):
        concat_in = [
            np.concatenate([m[nm] for m in in_maps], axis=0)
            for nm in in_names
        ]
        concat_zeros = [
            np.zeros((NCORES * av.shape[0], *av.shape[1:]), av.dtype)
            for av in out_avals
        ]
        return [jax.device_put(x, in_sharding)
                for x in concat_in + concat_zeros]

    def call(dev_args):
        outs = sharded(*dev_args)
        jax.block_until_ready(outs)
        return outs

    def call_async(dev_args):
        return sharded(*dev_args)

    def run(in_maps):
        outs = call(prep(in_maps))
        return {
            nm: np.asarray(outs[i]).reshape(NCORES, *out_avals[i].shape)[0]
            for i, nm in enumerate(out_names)
        }

    def make_chain(iters):
        # K sequential executions inside one jit call, serialized by
        # threading the output zero-buffer through each step — measures
        # on-device per-iteration time without tunnel dispatch overhead.
        def _chain(*args):
            ins = list(args[:n_params])
            state = list(args[n_params:])
            for _ in range(iters):
                state = list(_body(*ins, *state))
            return tuple(state)

        return jax.jit(
            shard_map(
                _chain, mesh=mesh,
                in_specs=(PartitionSpec("core"),) * (n_params + n_outs),
                out_specs=(PartitionSpec("core"),) * n_outs,
                check_rep=False,
            ),
            keep_unused=True,
        )

    run.prep = prep
    run.call = call
    run.call_async = call_async
    run.make_chain = make_chain
    _CACHE["runner"] = run
    return run


def _run_fast(output1, output2):
    run = _get_fast_runner()
    out = run(_in_maps(output1, output2))["out"]
    return np.asarray(out, dtype=np.float32).reshape(())



# revision 4
# speedup vs baseline: 7.2303x; 7.2303x over previous
"""ContrastiveLoss (margin=1) on 8 trn2 NeuronCores via Bass/Tile. v2.

Math (see kernel.py docstring): loss = (1 - 1/N) + sum(d)/N^2
  - (mean(pos_r) + mean(pos_c))/2, with d = cdist(output1, output2),
pos_r = rowmin(d), pos_c = colmin(d).  Kernel computes sum(d), rowmin,
colmin in one streaming pass over e2 = r1[i] + r2[j] - 2 b_j . a_i.

v2 layout (per core, core owns b-strip = 1024 rows of output2):
  partitions = j (8 blocks of 128), free = i (8192 in 4 groups of 2048).
  PSUM: one pool, 2 bufs x [128, 2048] f32 (8 banks total).
  Per (ig, jb): 4 main matmuls (lhsT = bT block, f32r) + 4 rank-1
  matmuls (ones x (-r1/2), f32r) -> ACT sqrt [128,2048] with scale=-2,
  bias=r2[jb], accum -> dsum; bf16 d tiles feed TT-min accumulation:
  rowacc[jb] (over ig) and colacc (over jb), split DVE/Pool.
  colacc finalized per ig via 16 XBAR dma transposes + 1 min-reduce.
  Cross-core: one AllReduce(min) over [128, 64+8] f32 (colmin_t +
  per-core combo scalars in masked lanes).
"""

import numpy as np
from contextlib import ExitStack

N = 8192          # rows of output1 == rows of output2
D = 128           # feature dim
NCORES = 8
R = N // NCORES   # 1024 rows of output2 per core
IG = 2048         # free-dim group (4 PSUM banks)
NIG = N // IG     # 4 groups
NIB = R // 128    # 8 j-blocks per core
CH = 512          # matmul tile width (1 PSUM bank)

MARGIN = 1.0
C0 = 1.0 / (float(N) * float(N))      # scale for sum(d)
C1 = -1.0 / (2.0 * float(N))          # scale for sum(pos_r)
C2 = -1.0 / (2.0 * float(N))          # scale for sum(pos_c)
CONST = MARGIN - MARGIN / float(N)    # 1 - 1/8192

# engine split for the min accumulations (jb indices owned by Pool/gpsimd)
POOL_ROW_JB = ()   # hw: gpsimd lacks TensorTensor-min; all mins on DVE

_CACHE = {}


def _build():
    import concourse.bass as bass
    import concourse.bacc as bacc
    import concourse.tile as tile
    from concourse import mybir
    from concourse import bass_isa

    f32 = mybir.dt.float32
    f32r = mybir.dt.float32r
    bf16 = mybir.dt.bfloat16
    X = mybir.AxisListType.X
    MIN = mybir.AluOpType.min
    ADD = mybir.AluOpType.add
    MULT = mybir.AluOpType.mult
    Sqrt = mybir.ActivationFunctionType.Sqrt
    Square = mybir.ActivationFunctionType.Square

    nc = bacc.Bacc(
        trn_type="TRN2",
        target_bir_lowering=False,
        debug=False,
        num_devices=NCORES,
    )

    a_ext = nc.dram_tensor("a", [N, D], f32, kind="ExternalInput")
    b_ext = nc.dram_tensor("b", [R, D], f32, kind="ExternalInput")
    cmask_ext = nc.dram_tensor("cmask", [1, NCORES], f32, kind="ExternalInput")
    out_ext = nc.dram_tensor("out", [1, 1], f32, kind="ExternalOutput")

    groups = [list(range(NCORES))]

    with tile.TileContext(nc) as tc, ExitStack() as ctx:
        const = ctx.enter_context(tc.tile_pool(name="const", bufs=1))
        big = ctx.enter_context(tc.tile_pool(name="big", bufs=1))
        sqp = ctx.enter_context(tc.tile_pool(name="sqp", bufs=2))
        dpool = ctx.enter_context(tc.tile_pool(name="dpool", bufs=5))
        capool = ctx.enter_context(tc.tile_pool(name="capool", bufs=2))
        catp = ctx.enter_context(tc.tile_pool(name="catp", bufs=2))
        mpsum = ctx.enter_context(tc.tile_pool(name="mpsum", bufs=2, space="PSUM"))
        dram = ctx.enter_context(tc.tile_pool(name="dram", bufs=1, space="DRAM"))

        # ---- constants ----
        id_dram = nc.inline_tensor(np.eye(128, dtype=np.float32), name="id128")
        identityd = const.tile([128, 128], f32)
        nc.sync.dma_start(out=identityd, in_=id_dram[:, :])
        identity = const.tile([128, 128], f32)
        nc.vector.tensor_copy(out=identity, in_=identityd)

        ones128f = const.tile([128, 1], f32)
        nc.vector.memset(ones128f, 1.0)
        ones128 = const.tile([128, 1], f32r)
        nc.scalar.copy(out=ones128, in_=ones128f)
        onesrf = const.tile([1, 128], f32)
        nc.vector.memset(onesrf, 1.0)
        ones_row = const.tile([1, 128], f32r)
        nc.scalar.copy(out=ones_row, in_=onesrf)

        # ---- input DMAs ----
        b_nat = big.tile([128, NIB, D], f32)
        nc.sync.dma_start(
            out=b_nat, in_=b_ext[:, :].rearrange("(q p) d -> p q d", p=128))
        a_nat = big.tile([128, N // 128, D], f32)
        QC = 16           # a-row blocks per prologue chunk (2048 rows)
        for q in range(NIG):
            nc.sync.dma_start(
                out=a_nat[:, q * QC:(q + 1) * QC, :],
                in_=a_ext[q * IG:(q + 1) * IG, :].rearrange(
                    "(q p) d -> p q d", p=128))

        # ---- persistent SBUF ----
        aT = big.tile([128, N], f32r)            # a^T, matmul rhs
        bT_sb = big.tile([128, R], f32r)         # b^T blocks, matmul lhsT
        sq_all = big.tile([128, NIG, IG], bf16)  # aT squares (r1 via matmul)
        r2_vec = const.tile([128, NIB], f32)     # per-partition sqrt bias
        rowacc = big.tile([128, NIB, IG], bf16)  # rowmin accumulators
        dsum_all = const.tile([128, NIG * NIB], f32)
        colmin_t = const.tile([128, N // 128], f32)
        rowmin8 = const.tile([128, NIB], f32)

        # ---- b-side prologue: bT + r2 ----
        bps = mpsum.tile([128, IG], f32, tag="mps")
        for k in range(NIB):
            nc.tensor.transpose(bps[:, k * 128:(k + 1) * 128],
                                b_nat[:, k, :], identity)
            scr = sqp.tile([128, D], f32, tag="scr")
            nc.vector.scalar_tensor_tensor(
                out=scr, in0=b_nat[:, k, :], scalar=1.0, in1=b_nat[:, k, :],
                op0=MULT, op1=MULT,
                accum_out=r2_vec[:, k:k + 1])
        # bT copy PSUM -> SBUF (f32 bits into f32r tile) via DVE
        nc.vector.tensor_copy(out=bT_sb, in_=bps[:, :R])

        # ---- interleaved prologue chunks + main loop ----
        colacc_tiles = {}

        def prologue_chunk(q):
            sl = slice(q * IG, (q + 1) * IG)
            # 16 transposes of a blocks into one 4-bank psum tile
            tp = mpsum.tile([128, IG], f32, tag="mps")
            for k in range(QC):
                nc.tensor.transpose(tp[:, k * 128:(k + 1) * 128],
                                    a_nat[:, q * QC + k, :], identity)
            # aT chunk: PSUM -> SBUF on ACT (gpsimd cannot access PSUM)
            nc.scalar.copy(out=aT[:, sl], in_=tp)
            # squares (bf16) on ACT; the -r1/2 term is added in the main loop
            # as a K=128 matmul of the constant -0.5 matrix against sq_all.
            nc.scalar.activation(out=sq_all[:, q, :], in_=tp, func=Square)

        # constant -0.5 matrix (bf16) for the r1-sum matmuls
        mhalf_f = const.tile([128, 128], f32)
        nc.vector.memset(mhalf_f, -0.5)
        mhalf = const.tile([128, 128], bf16)
        nc.vector.tensor_copy(out=mhalf, in_=mhalf_f)

        def main_group(ig):
            # two independent 4-step colmin chains (A: jb 0-3, B: jb 4-7),
            # both on DVE; A's XBAR transposes start mid-group.
            colA = capool.tile([128, IG], bf16, tag="colA")
            colB = capool.tile([128, IG], bf16, tag="colB")
            cat = catp.tile([128, IG // 128, 128], bf16, tag="cat")
            for jb in range(NIB):
                ps = mpsum.tile([128, IG], f32, tag="mps")
                wA = bT_sb[:, jb * 128:(jb + 1) * 128]
                for k in range(IG // CH):
                    nc.tensor.matmul(
                        ps[:, k * CH:(k + 1) * CH], lhsT=wA,
                        rhs=aT[:, ig * IG + k * CH:ig * IG + (k + 1) * CH],
                        start=True, stop=False)
                for k in range(IG // CH):
                    nc.tensor.matmul(
                        ps[:, k * CH:(k + 1) * CH], lhsT=mhalf,
                        rhs=sq_all[:, ig, k * CH:(k + 1) * CH],
                        start=False, stop=True)
                dsc = dpool.tile([128, IG], bf16, tag="dsc")
                s = ig * NIB + jb
                nc.scalar.activation(
                    out=dsc, in_=ps, func=Sqrt,
                    bias=r2_vec[:, jb:jb + 1], scale=-2.0,
                    accum_out=dsum_all[:, s:s + 1])
                # rowmin accumulation (over ig): Pool for latency-tolerant rows
                reng = nc.gpsimd if jb in POOL_ROW_JB else nc.vector
                if ig == 0:
                    reng.tensor_copy(out=rowacc[:, jb, :], in_=dsc)
                else:
                    reng.tensor_tensor(
                        out=rowacc[:, jb, :], in0=dsc, in1=rowacc[:, jb, :],
                        op=MIN)
                # colmin accumulation (over jb), DVE, chain A or B
                cacc = colA if jb < 4 else colB
                if jb % 4 == 0:
                    nc.vector.tensor_copy(out=cacc, in_=dsc)
                else:
                    nc.vector.tensor_tensor(out=cacc, in0=dsc, in1=cacc,
                                            op=MIN)
                if ig == NIG - 1:
                    # rowacc[jb] final: reduce along free (DVE only: gpsimd
                    # tensor_reduce is partition-axis only)
                    nc.vector.tensor_reduce(
                        out=rowmin8[:, jb:jb + 1], in_=rowacc[:, jb, :],
                        axis=X, op=MIN)
            # merge the two chains, then one XBAR pass over the merged tile
            nc.vector.tensor_tensor(out=colA, in0=colB, in1=colA, op=MIN)
            for blk in range(IG // 128):
                nc.sync.dma_start_transpose(
                    cat[:, blk, :], colA[:, blk * 128:(blk + 1) * 128])
            nc.vector.tensor_reduce(
                out=colmin_t[:, ig * (IG // 128):(ig + 1) * (IG // 128)],
                in_=cat, axis=X, op=MIN)

        # prefetch one chunk ahead so the next ig's aT/sq are ready when the
        # current group drains (chunk prep is a hard prereq for a whole ig)
        prologue_chunk(0)
        prologue_chunk(1)
        for q in range(NIG):
            main_group(q)
            if q + 2 < NIG:
                prologue_chunk(q + 2)

        # ---- local scalar stats ----
        dsum_vec = const.tile([128, 1], f32)
        nc.vector.tensor_reduce(out=dsum_vec, in_=dsum_all, axis=X, op=ADD)
        posc_vec = const.tile([128, 1], f32)
        nc.vector.tensor_reduce(out=posc_vec, in_=rowmin8, axis=X, op=ADD)
        dsum_sc = const.tile([128, 1], f32)
        nc.vector.tensor_scalar_mul(dsum_sc, dsum_vec, C0)
        combo_l = const.tile([128, 1], f32)
        nc.vector.scalar_tensor_tensor(
            out=combo_l, in0=posc_vec, scalar=C2, in1=dsum_sc,
            op0=MULT, op1=ADD)
        combo_all = const.tile([128, 1], f32)
        nc.gpsimd.partition_all_reduce(
            out_ap=combo_all, in_ap=combo_l, channels=128,
            reduce_op=bass_isa.ReduceOp.add)

        cmaskd = const.tile([128, NCORES], f32)
        nc.sync.dma_start(
            out=cmaskd, in_=cmask_ext[0:1, :].to_broadcast((128, NCORES)))
        cmx = const.tile([128, NCORES], f32)
        nc.vector.tensor_scalar_add(cmx, cmaskd, combo_all)

        # ---- single all-reduce(min): [colmin_t | combo slots] ----
        W = N // 128 + NCORES
        cm_in = dram.tile([128, W], f32)
        cm_out = dram.tile([128, W], f32)
        nc.sync.dma_start(out=cm_in[:, :N // 128], in_=colmin_t)
        nc.sync.dma_start(out=cm_in[:, N // 128:], in_=cmx)
        nc.gpsimd.collective_compute(
            "AllReduce", MIN, replica_groups=groups,
            ins=[cm_in.opt()], outs=[cm_out.opt()])
        colmin_g = const.tile([128, W], f32)
        nc.sync.dma_start(out=colmin_g, in_=cm_out)
        posr_vec = const.tile([128, 1], f32)
        nc.vector.tensor_reduce(
            out=posr_vec, in_=colmin_g[:, :N // 128], axis=X, op=ADD)
        combo_g = const.tile([128, 1], f32)
        nc.vector.tensor_reduce(
            out=combo_g, in_=colmin_g[:, N // 128:], axis=X, op=ADD)

        # ---- final combine: PAR over posr first, then add combo_g ----
        posr_sc = const.tile([128, 1], f32)
        nc.vector.tensor_scalar_mul(posr_sc, posr_vec, C1)
        pr = const.tile([128, 1], f32)
        nc.gpsimd.partition_all_reduce(
            out_ap=pr, in_ap=posr_sc, channels=128,
            reduce_op=bass_isa.ReduceOp.add)
        total_vec = const.tile([128, 1], f32)
        nc.vector.tensor_tensor(out=total_vec, in0=pr, in1=combo_g, op=ADD)
        fin = const.tile([1, 1], f32)
        cbias = const.tile([1, 1], f32)
        nc.vector.memset(cbias, CONST)
        nc.scalar.activation(
            out=fin, in_=total_vec[0:1, :],
            func=mybir.ActivationFunctionType.Identity,
            bias=cbias, scale=1.0)
        nc.sync.dma_start(out=out_ext[:], in_=fin)

    if not nc.is_finalized():
        nc.finalize()
    return nc


def _get_nc():
    if "nc" not in _CACHE:
        _CACHE["nc"] = _build()
    return _CACHE["nc"]


def _in_maps(output1, output2):
    a = np.ascontiguousarray(np.asarray(output1, dtype=np.float32))
    b = np.ascontiguousarray(np.asarray(output2, dtype=np.float32))
    assert a.shape == (N, D) and b.shape == (N, D)
    masks = np.full((NCORES, 1, NCORES), 1e30, dtype=np.float32)
    for c in range(NCORES):
        masks[c, 0, c] = 0.0
    return [{"a": a, "b": b[c * R:(c + 1) * R], "cmask": masks[c]}
            for c in range(NCORES)]


def _run(output1, output2, trace=False):
    from concourse.bass_utils import run_bass_kernel_spmd

    res = run_bass_kernel_spmd(
        _get_nc(), _in_maps(output1, output2), list(range(NCORES)), trace=trace)
    out = np.asarray(res.results[0]["out"], dtype=np.float32).reshape(())
    return out, res


def kernel(output1, output2):
    out, _ = _run(output1, output2, trace=False)
    return out


# ---------------------------------------------------------------------------
# cached fast runner (mirrors bass2jax.run_bass_via_pjrt, but keeps the
# jitted sharded callable alive so repeated calls don't re-trace) — used by
# test.py for warm timing loops.
def _get_fast_runner():
    if "runner" in _CACHE:
        return _CACHE["runner"]

    import jax
    from jax.experimental.shard_map import shard_map
    from jax.sharding import Mesh, PartitionSpec
    from concourse import bass2jax, mybir

    nc = _get_nc()
    bass2jax.install_neuronx_cc_hook()

    partition_name = (
        nc.partition_id_tensor.name if nc.partition_id_tensor else None)
    in_names, out_names, out_avals = [], [], []
    for alloc in nc.m.functions[0].allocations:
        if not isinstance(alloc, mybir.MemoryLocationSet):
            continue
        name = alloc.memorylocations[0].name
        if alloc.kind == "ExternalInput":
            if name != partition_name:
                in_names.append(name)
        elif alloc.kind == "ExternalOutput":
            out_names.append(name)
            out_avals.append(jax.core.ShapedArray(
                tuple(alloc.tensor_shape), mybir.dt.np(alloc.dtype)))
    n_params = len(in_names)
    all_in_names = list(in_names) + list(out_names)
    if partition_name is not None:
        all_in_names.append(partition_name)

    def _body(*args):
        operands = list(args)
        if partition_name is not None:
            operands.append(bass2jax.partition_id_tensor())
        return tuple(bass2jax._bass_exec_p.bind(
            *operands,
            out_avals=tuple(out_avals),
            in_names=tuple(all_in_names),
            out_names=tuple(out_names),
            lowering_input_output_aliases=(),
            sim_require_finite=True,
            sim_require_nnan=True,
            nc=nc,
        ))

    devices = jax.devices()[:NCORES]
    mesh = Mesh(np.asarray(devices), ("core",))
    n_outs = len(out_names)
    sharded = jax.jit(
        shard_map(
            _body, mesh=mesh,
            in_specs=(PartitionSpec("core"),) * (n_params + n_outs),
            out_specs=(PartitionSpec("core"),) * n_outs,
            check_rep=False,
        ),
        keep_unused=True,
    )

    in_sharding = jax.sharding.NamedSharding(mesh, PartitionSpec("core"))

    def prep(in_maps):
        concat_in = [
            np.concatenate([m[nm] for m in in_maps], axis=0)
            for nm in in_names
        ]
        concat_zeros = [
            np.zeros((NCORES * av.shape[0], *av.shape[1:]), av.dtype)
            for av in out_avals
        ]
        return [jax.device_put(x, in_sharding)
                for x in concat_in + concat_zeros]

    def call(dev_args):
        outs = sharded(*dev_args)
        jax.block_until_ready(outs)
        return outs

    def call_async(dev_args):
        return sharded(*dev_args)

    def run(in_maps):
        outs = call(prep(in_maps))
        return {
            nm: np.asarray(outs[i]).reshape(NCORES, *out_avals[i].shape)[0]
            for i, nm in enumerate(out_names)
        }

    def make_chain(iters):
        # K sequential executions inside one jit call, serialized by
        # threading the output zero-buffer through each step — measures
        # on-device per-iteration time without tunnel dispatch overhead.
        def _chain(*args):
            ins = list(args[:n_params])
            state = list(args[n_params:])
            for _ in range(iters):
                state = list(_body(*ins, *state))
            return tuple(state)

        return jax.jit(
            shard_map(
                _chain, mesh=mesh,
                in_specs=(PartitionSpec("core"),) * (n_params + n_outs),
                out_specs=(PartitionSpec("core"),) * n_outs,
                check_rep=False,
            ),
            keep_unused=True,
        )

    run.prep = prep
    run.call = call
    run.call_async = call_async
    run.make_chain = make_chain
    _CACHE["runner"] = run
    return run


def _run_fast(output1, output2):
    run = _get_fast_runner()
    out = run(_in_maps(output1, output2))["out"]
    return np.asarray(out, dtype=np.float32).reshape(())



# revision 6
# speedup vs baseline: 8.1788x; 1.1312x over previous
"""ContrastiveLoss (margin=1) on 8 trn2 NeuronCores via Bass/Tile. v2.

Math (see kernel.py docstring): loss = (1 - 1/N) + sum(d)/N^2
  - (mean(pos_r) + mean(pos_c))/2, with d = cdist(output1, output2),
pos_r = rowmin(d), pos_c = colmin(d).  Kernel computes sum(d), rowmin,
colmin in one streaming pass over e2 = r1[i] + r2[j] - 2 b_j . a_i.

v2 layout (per core, core owns b-strip = 1024 rows of output2):
  partitions = j (8 blocks of 128), free = i (8192 in 4 groups of 2048).
  PSUM: one pool, 2 bufs x [128, 2048] f32 (8 banks total).
  Per (ig, jb): 4 main matmuls (lhsT = bT block, f32r) + 4 rank-1
  matmuls (ones x (-r1/2), f32r) -> ACT sqrt [128,2048] with scale=-2,
  bias=r2[jb], accum -> dsum; bf16 d tiles feed TT-min accumulation:
  rowacc[jb] (over ig) and colacc (over jb), split DVE/Pool.
  colacc finalized per ig via 16 XBAR dma transposes + 1 min-reduce.
  Cross-core: one AllReduce(min) over [128, 64+8] f32 (colmin_t +
  per-core combo scalars in masked lanes).
"""

import numpy as np
from contextlib import ExitStack

N = 8192          # rows of output1 == rows of output2
D = 128           # feature dim
NCORES = 8
R = N // NCORES   # 1024 rows of output2 per core
IG = 2048         # free-dim group (4 PSUM banks)
NIG = N // IG     # 4 groups
NIB = R // 128    # 8 j-blocks per core
CH = 512          # matmul tile width (1 PSUM bank)

MARGIN = 1.0
C0 = 1.0 / (float(N) * float(N))      # scale for sum(d)
C1 = -1.0 / (2.0 * float(N))          # scale for sum(pos_r)
C2 = -1.0 / (2.0 * float(N))          # scale for sum(pos_c)
CONST = MARGIN - MARGIN / float(N)    # 1 - 1/8192

# engine split for the min accumulations (jb indices owned by Pool/gpsimd)
POOL_ROW_JB = ()   # hw: gpsimd lacks TensorTensor-min; all mins on DVE

_CACHE = {}


def _build():
    import concourse.bass as bass
    import concourse.bacc as bacc
    import concourse.tile as tile
    from concourse import mybir
    from concourse import bass_isa

    f32 = mybir.dt.float32
    f32r = mybir.dt.float32r
    bf16 = mybir.dt.bfloat16
    X = mybir.AxisListType.X
    MIN = mybir.AluOpType.min
    ADD = mybir.AluOpType.add
    MULT = mybir.AluOpType.mult
    Sqrt = mybir.ActivationFunctionType.Sqrt
    Square = mybir.ActivationFunctionType.Square

    nc = bacc.Bacc(
        trn_type="TRN2",
        target_bir_lowering=False,
        debug=False,
        num_devices=NCORES,
    )

    a_ext = nc.dram_tensor("a", [N, D], f32, kind="ExternalInput")
    b_ext = nc.dram_tensor("b", [R, D], f32, kind="ExternalInput")
    cmask_ext = nc.dram_tensor("cmask", [1, NCORES], f32, kind="ExternalInput")
    out_ext = nc.dram_tensor("out", [1, 1], f32, kind="ExternalOutput")

    groups = [list(range(NCORES))]

    with tile.TileContext(nc) as tc, ExitStack() as ctx:
        const = ctx.enter_context(tc.tile_pool(name="const", bufs=1))
        big = ctx.enter_context(tc.tile_pool(name="big", bufs=1))
        sqp = ctx.enter_context(tc.tile_pool(name="sqp", bufs=2))
        dpool = ctx.enter_context(tc.tile_pool(name="dpool", bufs=5))
        capool = ctx.enter_context(tc.tile_pool(name="capool", bufs=2))
        catp = ctx.enter_context(tc.tile_pool(name="catp", bufs=2))
        mpsum = ctx.enter_context(tc.tile_pool(name="mpsum", bufs=2, space="PSUM"))
        dram = ctx.enter_context(tc.tile_pool(name="dram", bufs=1, space="DRAM"))

        # ---- constants ----
        id_dram = nc.inline_tensor(np.eye(128, dtype=np.float32), name="id128")
        identityd = const.tile([128, 128], f32)
        nc.sync.dma_start(out=identityd, in_=id_dram[:, :])
        identity = const.tile([128, 128], f32)
        nc.vector.tensor_copy(out=identity, in_=identityd)

        ones128f = const.tile([128, 1], f32)
        nc.vector.memset(ones128f, 1.0)
        ones128 = const.tile([128, 1], f32r)
        nc.scalar.copy(out=ones128, in_=ones128f)
        onesrf = const.tile([1, 128], f32)
        nc.vector.memset(onesrf, 1.0)
        ones_row = const.tile([1, 128], f32r)
        nc.scalar.copy(out=ones_row, in_=onesrf)

        # ---- input DMAs ----
        b_nat = big.tile([128, NIB, D], f32)
        nc.sync.dma_start(
            out=b_nat, in_=b_ext[:, :].rearrange("(q p) d -> p q d", p=128))
        a_nat = big.tile([128, N // 128, D], f32)
        QC = 16           # a-row blocks per prologue chunk (2048 rows)
        for q in range(NIG):
            nc.sync.dma_start(
                out=a_nat[:, q * QC:(q + 1) * QC, :],
                in_=a_ext[q * IG:(q + 1) * IG, :].rearrange(
                    "(q p) d -> p q d", p=128))

        # ---- persistent SBUF ----
        aT = big.tile([128, N], f32r)            # a^T, matmul rhs
        bT_sb = big.tile([128, R], f32r)         # b^T blocks, matmul lhsT
        sq_all = big.tile([128, NIG, IG], bf16)  # aT squares (r1 via matmul)
        r2_vec = const.tile([128, NIB], f32)     # per-partition sqrt bias
        rowacc = big.tile([128, NIB, IG], bf16)  # rowmin accumulators
        dsum_all = const.tile([128, NIG * NIB], f32)
        colmin_t = const.tile([128, N // 128], f32)
        rowmin8 = const.tile([128, NIB], f32)

        # ---- b-side prologue: bT + r2 ----
        bps = mpsum.tile([128, IG], f32, tag="mps")
        for k in range(NIB):
            nc.tensor.transpose(bps[:, k * 128:(k + 1) * 128],
                                b_nat[:, k, :], identity)
            scr = sqp.tile([128, D], f32, tag="scr")
            nc.vector.scalar_tensor_tensor(
                out=scr, in0=b_nat[:, k, :], scalar=1.0, in1=b_nat[:, k, :],
                op0=MULT, op1=MULT,
                accum_out=r2_vec[:, k:k + 1])
        # bT copy PSUM -> SBUF (f32 bits into f32r tile) via DVE
        nc.vector.tensor_copy(out=bT_sb, in_=bps[:, :R])

        # ---- interleaved prologue chunks + main loop ----
        colacc_tiles = {}

        def prologue_chunk(q):
            sl = slice(q * IG, (q + 1) * IG)
            # 16 transposes of a blocks into one 4-bank psum tile
            tp = mpsum.tile([128, IG], f32, tag="mps")
            for k in range(QC):
                nc.tensor.transpose(tp[:, k * 128:(k + 1) * 128],
                                    a_nat[:, q * QC + k, :], identity)
            # aT chunk: PSUM -> SBUF on ACT (gpsimd cannot access PSUM)
            nc.scalar.copy(out=aT[:, sl], in_=tp)
            # squares (bf16) on ACT; the -r1/2 term is added in the main loop
            # as a K=128 matmul of the constant -0.5 matrix against sq_all.
            nc.scalar.activation(out=sq_all[:, q, :], in_=tp, func=Square)

        # constant -0.5 matrix (bf16) for the r1-sum matmuls
        mhalf_f = const.tile([128, 128], f32)
        nc.vector.memset(mhalf_f, -0.5)
        mhalf = const.tile([128, 128], bf16)
        nc.vector.tensor_copy(out=mhalf, in_=mhalf_f)

        def main_group(ig):
            # two independent 4-step colmin chains (A: jb 0-3, B: jb 4-7),
            # both on DVE; A's XBAR transposes start mid-group.
            colA = capool.tile([128, IG], bf16, tag="colA")
            colB = capool.tile([128, IG], bf16, tag="colB")
            cat = catp.tile([128, IG // 128, 128], bf16, tag="cat")
            for jb in range(NIB):
                ps = mpsum.tile([128, IG], f32, tag="mps")
                wA = bT_sb[:, jb * 128:(jb + 1) * 128]
                for k in range(IG // CH):
                    nc.tensor.matmul(
                        ps[:, k * CH:(k + 1) * CH], lhsT=wA,
                        rhs=aT[:, ig * IG + k * CH:ig * IG + (k + 1) * CH],
                        start=True, stop=False)
                for k in range(IG // CH):
                    nc.tensor.matmul(
                        ps[:, k * CH:(k + 1) * CH], lhsT=mhalf,
                        rhs=sq_all[:, ig, k * CH:(k + 1) * CH],
                        start=False, stop=True)
                dsc = dpool.tile([128, IG], bf16, tag="dsc")
                s = ig * NIB + jb
                nc.scalar.activation(
                    out=dsc, in_=ps, func=Sqrt,
                    bias=r2_vec[:, jb:jb + 1], scale=-2.0,
                    accum_out=dsum_all[:, s:s + 1])
                # rowmin accumulation (over ig)
                if ig == 0:
                    nc.vector.tensor_copy(out=rowacc[:, jb, :], in_=dsc)
                else:
                    nc.vector.tensor_tensor(
                        out=rowacc[:, jb, :], in0=dsc, in1=rowacc[:, jb, :],
                        op=MIN)
                if ig == NIG - 1:
                    nc.vector.tensor_reduce(
                        out=rowmin8[:, jb:jb + 1], in_=rowacc[:, jb, :],
                        axis=X, op=MIN)
                # colmin accumulation (over jb), DVE, chain A or B
                cacc = colA if jb < 4 else colB
                if jb % 4 == 0:
                    nc.vector.tensor_copy(out=cacc, in_=dsc)
                else:
                    nc.vector.tensor_tensor(out=cacc, in0=dsc, in1=cacc,
                                            op=MIN)
            # merge the two chains, then one XBAR pass over the merged tile
            nc.vector.tensor_tensor(out=colA, in0=colB, in1=colA, op=MIN)
            for blk in range(IG // 128):
                nc.sync.dma_start_transpose(
                    cat[:, blk, :], colA[:, blk * 128:(blk + 1) * 128])
            nc.vector.tensor_reduce(
                out=colmin_t[:, ig * (IG // 128):(ig + 1) * (IG // 128)],
                in_=cat, axis=X, op=MIN)

        # prefetch one chunk ahead so the next ig's aT/sq are ready when the
        # current group drains (chunk prep is a hard prereq for a whole ig)
        prologue_chunk(0)
        prologue_chunk(1)
        for q in range(NIG):
            main_group(q)
            if q + 2 < NIG:
                prologue_chunk(q + 2)

        # ---- local scalar stats ----
        dsum_vec = const.tile([128, 1], f32)
        nc.vector.tensor_reduce(out=dsum_vec, in_=dsum_all, axis=X, op=ADD)
        posc_vec = const.tile([128, 1], f32)
        nc.vector.tensor_reduce(out=posc_vec, in_=rowmin8, axis=X, op=ADD)
        dsum_sc = const.tile([128, 1], f32)
        nc.vector.tensor_scalar_mul(dsum_sc, dsum_vec, C0)
        combo_l = const.tile([128, 1], f32)
        nc.vector.scalar_tensor_tensor(
            out=combo_l, in0=posc_vec, scalar=C2, in1=dsum_sc,
            op0=MULT, op1=ADD)
        combo_all = const.tile([128, 1], f32)
        nc.gpsimd.partition_all_reduce(
            out_ap=combo_all, in_ap=combo_l, channels=128,
            reduce_op=bass_isa.ReduceOp.add)

        cmaskd = const.tile([128, NCORES], f32)
        nc.sync.dma_start(
            out=cmaskd, in_=cmask_ext[0:1, :].to_broadcast((128, NCORES)))
        cmx = const.tile([128, NCORES], f32)
        nc.vector.tensor_scalar_add(cmx, cmaskd, combo_all)

        # ---- single all-reduce(min): [colmin_t | combo slots] ----
        W = N // 128 + NCORES
        cm_in = dram.tile([128, W], f32)
        cm_out = dram.tile([128, W], f32)
        nc.sync.dma_start(out=cm_in[:, :N // 128], in_=colmin_t)
        nc.sync.dma_start(out=cm_in[:, N // 128:], in_=cmx)
        nc.gpsimd.collective_compute(
            "AllReduce", MIN, replica_groups=groups,
            ins=[cm_in.opt()], outs=[cm_out.opt()])
        colmin_g = const.tile([128, W], f32)
        nc.sync.dma_start(out=colmin_g, in_=cm_out)
        posr_vec = const.tile([128, 1], f32)
        nc.vector.tensor_reduce(
            out=posr_vec, in_=colmin_g[:, :N // 128], axis=X, op=ADD)
        combo_g = const.tile([128, 1], f32)
        nc.vector.tensor_reduce(
            out=combo_g, in_=colmin_g[:, N // 128:], axis=X, op=ADD)

        # ---- final combine: PAR over posr first, then add combo_g ----
        posr_sc = const.tile([128, 1], f32)
        nc.vector.tensor_scalar_mul(posr_sc, posr_vec, C1)
        pr = const.tile([128, 1], f32)
        nc.gpsimd.partition_all_reduce(
            out_ap=pr, in_ap=posr_sc, channels=128,
            reduce_op=bass_isa.ReduceOp.add)
        total_vec = const.tile([128, 1], f32)
        nc.vector.tensor_tensor(out=total_vec, in0=pr, in1=combo_g, op=ADD)
        fin = const.tile([1, 1], f32)
        cbias = const.tile([1, 1], f32)
        nc.vector.memset(cbias, CONST)
        nc.scalar.activation(
            out=fin, in_=total_vec[0:1, :],
            func=mybir.ActivationFunctionType.Identity,
            bias=cbias, scale=1.0)
        nc.sync.dma_start(out=out_ext[:], in_=fin)

    if not nc.is_finalized():
        nc.finalize()
    return nc


def _get_nc():
    if "nc" not in _CACHE:
        _CACHE["nc"] = _build()
    return _CACHE["nc"]


def _in_maps(output1, output2):
    a = np.ascontiguousarray(np.asarray(output1, dtype=np.float32))
    b = np.ascontiguousarray(np.asarray(output2, dtype=np.float32))
    assert a.shape == (N, D) and b.shape == (N, D)
    masks = np.full((NCORES, 1, NCORES), 1e30, dtype=np.float32)
    for c in range(NCORES):
        masks[c, 0, c] = 0.0
    return [{"a": a, "b": b[c * R:(c + 1) * R], "cmask": masks[c]}
            for c in range(NCORES)]


def _run(output1, output2, trace=False):
    from concourse.bass_utils import run_bass_kernel_spmd

    res = run_bass_kernel_spmd(
        _get_nc(), _in_maps(output1, output2), list(range(NCORES)), trace=trace)
    out = np.asarray(res.results[0]["out"], dtype=np.float32).reshape(())
    return out, res


def kernel(output1, output2):
    out, _ = _run(output1, output2, trace=False)
    return out


# ---------------------------------------------------------------------------
# cached fast runner (mirrors bass2jax.run_bass_via_pjrt, but keeps the
# jitted sharded callable alive so repeated calls don't re-trace) — used by
# test.py for warm timing loops.
def _get_fast_runner():
    if "runner" in _CACHE:
        return _CACHE["runner"]

    import jax
    from jax.experimental.shard_map import shard_map
    from jax.sharding import Mesh, PartitionSpec
    from concourse import bass2jax, mybir

    nc = _get_nc()
    bass2jax.install_neuronx_cc_hook()

    partition_name = (
        nc.partition_id_tensor.name if nc.partition_id_tensor else None)
    in_names, out_names, out_avals = [], [], []
    for alloc in nc.m.functions[0].allocations:
        if not isinstance(alloc, mybir.MemoryLocationSet):
            continue
        name = alloc.memorylocations[0].name
        if alloc.kind == "ExternalInput":
            if name != partition_name:
                in_names.append(name)
        elif alloc.kind == "ExternalOutput":
            out_names.append(name)
            out_avals.append(jax.core.ShapedArray(
                tuple(alloc.tensor_shape), mybir.dt.np(alloc.dtype)))
    n_params = len(in_names)
    all_in_names = list(in_names) + list(out_names)
    if partition_name is not None:
        all_in_names.append(partition_name)

    def _body(*args):
        operands = list(args)
        if partition_name is not None:
            operands.append(bass2jax.partition_id_tensor())
        return tuple(bass2jax._bass_exec_p.bind(
            *operands,
            out_avals=tuple(out_avals),
            in_names=tuple(all_in_names),
            out_names=tuple(out_names),
            lowering_input_output_aliases=(),
            sim_require_finite=True,
            sim_require_nnan=True,
            nc=nc,
        ))

    devices = jax.devices()[:NCORES]
    mesh = Mesh(np.asarray(devices), ("core",))
    n_outs = len(out_names)
    sharded = jax.jit(
        shard_map(
            _body, mesh=mesh,
            in_specs=(PartitionSpec("core"),) * (n_params + n_outs),
            out_specs=(PartitionSpec("core"),) * n_outs,
            check_rep=False,
        ),
        keep_unused=True,
    )

    in_sharding = jax.sharding.NamedSharding(mesh, PartitionSpec("core"))

    def prep(in_maps):
        concat_in = [
            np.concatenate([m[nm] for m in in_maps], axis=0)
            for nm in in_names
        ]
        concat_zeros = [
            np.zeros((NCORES * av.shape[0], *av.shape[1:]), av.dtype)
            for av in out_avals
        ]
        return [jax.device_put(x, in_sharding)
                for x in concat_in + concat_zeros]

    def call(dev_args):
        outs = sharded(*dev_args)
        jax.block_until_ready(outs)
        return outs

    def call_async(dev_args):
        return sharded(*dev_args)

    def run(in_maps):
        outs = call(prep(in_maps))
        return {
            nm: np.asarray(outs[i]).reshape(NCORES, *out_avals[i].shape)[0]
            for i, nm in enumerate(out_names)
        }

    def make_chain(iters):
        # K sequential executions inside one jit call, serialized by
        # threading the output zero-buffer through each step — measures
        # on-device per-iteration time without tunnel dispatch overhead.
        def _chain(*args):
            ins = list(args[:n_params])
            state = list(args[n_params:])
            for _ in range(iters):
                state = list(_body(*ins, *state))
            return tuple(state)

        return jax.jit(
            shard_map(
                _chain, mesh=mesh,
                in_specs=(PartitionSpec("core"),) * (n_params + n_outs),
                out_specs=(PartitionSpec("core"),) * n_outs,
                check_rep=False,
            ),
            keep_unused=True,
        )

    run.prep = prep
    run.call = call
    run.call_async = call_async
    run.make_chain = make_chain
    _CACHE["runner"] = run
    return run


def _run_fast(output1, output2):
    run = _get_fast_runner()
    out = run(_in_maps(output1, output2))["out"]
    return np.asarray(out, dtype=np.float32).reshape(())

